# revision 1
# baseline (speedup 1.0000x reference)
"""CrossViewRegionAligner Trainium2 kernel.

Computes the pairwise-MLP similarity logits on 8 NeuronCores (sat-axis
sharded, 192 rows/core), then performs the sequential greedy bipartite
matching on host (it is O(N^2) scalar work on a [1536,1536] matrix and
inherently sequential).

Device math per core, all fp32 (bf16/fp16 flips greedy argmax picks --
verified empirically; min runner-up logit gap is 6.3e-5):
  c_i      = sat_i @ W1[:64] + b1                      (PE, once)
  hu_j     = uav_j @ W1[64:]                           (PE, once, kept
             replicated 2x across partitions in SBUF: [128, 1536])
  h1(i,j)  = relu(hu_j + c_i)        per 2 sat rows    (DVE tensor_scalar
             fused add+max, per-partition scalar = packed c pair)
  h2(i,j)  = relu(blockdiag(W2,W2)^T h1 + b2)          (PE K=128 -> PSUM,
             ACT fused relu+bias PSUM->SBUF, 4 rows/instr)
  logit    = blockdiag(W3 x4)^T h2                     (PE, 16 passes
             accumulated into one PSUM tile via shifted zero-padded
             weight variants so 64 logit rows pack into 64 partitions)
"""

import os

import numpy as np

# If the axon NTFF profile hook is unavailable, a BASS_TRACE=1 environment
# would crash run_bass_kernel_spmd with ModuleNotFoundError -- disable
# tracing only in that case.
try:
    from antenv import axon_hooks as _axon_hooks  # noqa: F401
except Exception:
    os.environ.setdefault("BASS_NEVER_TRACE", "1")

import concourse.bass as bass
import concourse.bacc as bacc
import concourse.mybir as mybir
from concourse.tile import TileContext, add_dep_helper
from concourse.bass_utils import run_bass_kernel_spmd

FP = mybir.dt.float32
N_SAT = 1536
N_UAV = 1536
D = 64
N_CORES = 8
RPC = N_SAT // N_CORES          # sat rows per core = 192
PASSES = RPC // 4               # 4 sat rows per pass = 48
GROUP = 8                       # passes per logit-accumulation group (32 rows)
N_GROUPS = PASSES // GROUP      # 3
CHUNK = 512                     # matmul free-dim chunk (one PSUM bank, fp32)
N_CHUNKS = N_UAV // CHUNK       # 3

OFF_SE = N_UAV                      # 1536
OFF_SO = OFF_SE + RPC // 2          # 1632
OFF_W1BB = OFF_SO + RPC // 2        # 1728
OFF_W1ALO = OFF_W1BB + 128          # 1856
OFF_W1AHI = OFF_W1ALO + 128         # 1984
OFF_W2P = OFF_W1AHI + 128           # 2112
OFF_W3P = OFF_W2P + 64              # 2176
OFF_B2P = OFF_W3P + GROUP * 32      # 2432
BLOB_W = OFF_B2P + 16               # 2448

_CACHED_NC = None
LAST_RESULT = None  # BassKernelResults of the most recent run (for profiling)


def _build_nc():
    nc = bacc.Bacc(trn_type="TRN2")

    blob = nc.dram_tensor("blob", [128, BLOB_W], FP, kind="ExternalInput")
    lout_all = nc.dram_tensor(
        "lout_all", [GROUP * 4, N_GROUPS * N_CHUNKS * CHUNK], FP,
        kind="ExternalOutput",
    )

    with TileContext(nc) as tc:
        _body(nc, tc, blob, lout_all)
    nc.finalize()
    return nc


def _body(nc, tc, blob, lout_all):
    from contextlib import ExitStack

    H1S = 4    # h1 ring slots (per a/b buffer)
    H2S = 10   # h2 ring slots

    with ExitStack() as ctx:
        consts = ctx.enter_context(tc.tile_pool(name="consts", bufs=1))
        psum = ctx.enter_context(tc.tile_pool(name="psum", bufs=1, space="PSUM"))

        # ---- load all constants as ONE blob (one DMA => one queue sem) ---
        blob_sb = consts.tile([128, BLOB_W], FP, tag="blob")
        nc.gpsimd.dma_start(blob_sb[:], blob[:])
        uavT_sb = blob_sb[0:D, 0:N_UAV]
        satTe_sb = blob_sb[0 : D + 1, OFF_SE : OFF_SE + RPC // 2]
        satTo_sb = blob_sb[0 : D + 1, OFF_SO : OFF_SO + RPC // 2]
        w1bb_sb = blob_sb[0:D, OFF_W1BB : OFF_W1BB + 128]
        w1alo_sb = blob_sb[0 : D + 1, OFF_W1ALO : OFF_W1ALO + 128]
        w1ahi_sb = blob_sb[0 : D + 1, OFF_W1AHI : OFF_W1AHI + 128]
        w2p_sb = blob_sb[0:128, OFF_W2P : OFF_W2P + 64]
        w3p_sb = blob_sb[0:128, OFF_W3P : OFF_W3P + GROUP * 32]
        b2p_sb = blob_sb[0:128, OFF_B2P : OFF_B2P + 1]

        # Permanent PSUM tiles (no pool recycling => no slot-transition
        # multi-waits; same-engine WAW is program order).
        psA = psum.tile([128, N_UAV], FP, tag="psA")
        psB = psum.tile([128, N_UAV], FP, tag="psB")
        lpA = psum.tile([GROUP * 4, CHUNK], FP, tag="lpA")
        lpB = psum.tile([GROUP * 4, CHUNK], FP, tag="lpB")

        # Consume the blob DMA-queue semaphore once on PE and once on ACT
        # (ISA instructions hold a single sync wait).
        nc.tensor.matmul(
            psA[0:128, 0:1],
            blob_sb[:, OFF_W2P : OFF_W2P + 128],
            blob_sb[:, OFF_W2P : OFF_W2P + 1],
            skip_group_check=True,
        )
        b2probe = consts.tile([128, 1], FP, tag="b2probe")
        nc.scalar.copy(b2probe[:], b2p_sb[:])

        # ---- h_uav replicated [128, 1536] into psA, copy to SBUF ----------
        for c in range(N_CHUNKS):
            nc.tensor.matmul(
                psA[:, c * CHUNK : (c + 1) * CHUNK],
                w1bb_sb[:],
                uavT_sb[:, c * CHUNK : (c + 1) * CHUNK],
                skip_group_check=True,
            )
        huav_sb = consts.tile([128, N_UAV], FP, tag="huav")
        nc.vector.tensor_copy(huav_sb[:], psA[:])

        # ---- c_pack [128, 96] into psB: col t = (c_{2t} | c_{2t+1}) + b1 --
        nc.tensor.matmul(
            psB[:, : RPC // 2], w1alo_sb[:], satTe_sb[:], start=True, stop=False
        )
        nc.tensor.matmul(
            psB[:, : RPC // 2], w1ahi_sb[:], satTo_sb[:], start=False, stop=True
        )
        c_pack = consts.tile([128, RPC // 2], FP, tag="cpack")
        nc.vector.tensor_copy(c_pack[:], psB[:, : RPC // 2])

        # Permanent SBUF rings
        h1A = consts.tile([128, H1S * N_UAV], FP, tag="h1A")
        h1B = consts.tile([128, H1S * N_UAV], FP, tag="h1B")
        h2buf = consts.tile([128, H2S * N_UAV], FP, tag="h2buf")
        lsb_all = consts.tile(
            [GROUP * 4, N_GROUPS * N_CHUNKS * CHUNK], FP, tag="lsb_all"
        )

        # ---- main loop -----------------------------------------------------
        # Wait staircase: every ISA instruction may carry at most ONE sem
        # wait, but each instruction's transitive dependency closure spans
        # all engines. Per pass, each engine consumes the foreign-engine
        # clock ticks one at a time through dedicated catch-up ops, ordered
        # so that each op needs exactly one new semaphore value:
        #   DVE: TS-a {PE}, TS-b {PE}, flag {ACT}
        #   ACT: junkA-O {ACT own}, junkA-D {DVE}, act {PE}
        #   PE:  opener {ACT}, mm2 x6 {DVE}, (mm3 {ACT}, lsb-copy {PE})
        w2c = w2p_sb[:, 0:1]
        prev = {}  # last emitted instruction per engine, for chain edges

        def chain(key, binst):
            if key in prev:
                add_dep_helper(binst.ins, prev[key].ins, sync=False, reason="chain")
            prev[key] = binst
            return binst

        for g in range(N_GROUPS):
            for q in range(GROUP):
                t = g * GROUP + q  # pass index; sat rows 4t..4t+3
                h1o = (t % H1S) * N_UAV
                h2o = (t % H2S) * N_UAV
                # --- DVE ---
                chain("v", nc.vector.tensor_scalar(
                    out=h1A[:, h1o : h1o + N_UAV],
                    in0=huav_sb[:],
                    scalar1=c_pack[:, 2 * t : 2 * t + 1],
                    scalar2=0.0,
                    op0=mybir.AluOpType.add,
                    op1=mybir.AluOpType.max,
                ))
                if t % 10 == 9:
                    # balance engines: ACT computes this pass's h1B via its
                    # native fused relu(x + bias)
                    chain("a", nc.scalar.activation(
                        h1B[:, h1o : h1o + N_UAV],
                        huav_sb[:],
                        mybir.ActivationFunctionType.Relu,
                        bias=c_pack[:, 2 * t + 1 : 2 * t + 2],
                    ))
                else:
                    chain("v", nc.vector.tensor_scalar(
                        out=h1B[:, h1o : h1o + N_UAV],
                        in0=huav_sb[:],
                        scalar1=c_pack[:, 2 * t + 1 : 2 * t + 2],
                        scalar2=0.0,
                        op0=mybir.AluOpType.add,
                        op1=mybir.AluOpType.max,
                    ))
                # --- PE: opener + L2 matmuls ---
                ps = psA if t % 2 == 0 else psB
                chain("p", nc.tensor.matmul(
                    ps[0:1, 0:1], w2c, w2c, skip_group_check=True
                ))
                for c in range(N_CHUNKS):
                    sl = slice(c * CHUNK, (c + 1) * CHUNK)
                    hslc = slice(h1o + c * CHUNK, h1o + (c + 1) * CHUNK)
                    chain("p", nc.tensor.matmul(ps[0:64, sl], w2p_sb[:], h1A[:, hslc]))
                    chain("p", nc.tensor.matmul(ps[64:128, sl], w2p_sb[:], h1B[:, hslc]))
                # --- ACT staircase + L2 activation ---
                chain("a", nc.scalar.activation(
                    h2buf[:, h2o : h2o + N_UAV],
                    ps[:],
                    mybir.ActivationFunctionType.Relu,
                    bias=b2p_sb[:],
                ))

            # --- L3: accumulate GROUP passes into one PSUM tile ---
            for c in range(N_CHUNKS):
                sl = slice(c * CHUNK, (c + 1) * CHUNK)
                n = g * N_CHUNKS + c
                lp = lpA if n % 2 == 0 else lpB
                for q in range(GROUP):
                    p = g * GROUP + q
                    ho = (p % H2S) * N_UAV + c * CHUNK
                    chain("p", nc.tensor.matmul(
                        lp[:],
                        w3p_sb[:, q * 32 : (q + 1) * 32],
                        h2buf[:, ho : ho + CHUNK],
                        start=(q == 0),
                        stop=(q == GROUP - 1),
                    ))
                chain("a", nc.scalar.copy(
                    lsb_all[:, n * CHUNK : (n + 1) * CHUNK], lp[:]
                ))

        # single tail DMA of all logits (Bacc splits the drain's waits)
        nc.sync.dma_start(lout_all[:], lsb_all[:])


def _prepack(sat_shard, uav_regions, W1, b1, W2, b2, W3):
    f32 = np.float32
    W1a, W1b = W1[:D], W1[D:]

    blob = np.zeros((128, BLOB_W), f32)
    blob[0:D, 0:N_UAV] = uav_regions.T
    se = sat_shard[0::2].T.astype(f32)  # [64, 96]
    so = sat_shard[1::2].T.astype(f32)
    blob[0:D, OFF_SE : OFF_SE + RPC // 2] = se
    blob[D, OFF_SE : OFF_SE + RPC // 2] = 1.0
    blob[0:D, OFF_SO : OFF_SO + RPC // 2] = so
    blob[D, OFF_SO : OFF_SO + RPC // 2] = 1.0
    blob[0:D, OFF_W1BB : OFF_W1BB + 128] = np.hstack([W1b, W1b])
    blob[0:D, OFF_W1ALO : OFF_W1ALO + 64] = W1a
    blob[D, OFF_W1ALO : OFF_W1ALO + 64] = b1
    blob[0:D, OFF_W1AHI + 64 : OFF_W1AHI + 128] = W1a
    blob[D, OFF_W1AHI + 64 : OFF_W1AHI + 128] = b1
    blob[0:D, OFF_W2P : OFF_W2P + 32] = W2
    blob[D:128, OFF_W2P + 32 : OFF_W2P + 64] = W2
    # variant q places blockdiag(W3 x4) rows at output columns 4q..4q+3
    for q in range(GROUP):
        for r in range(4):
            blob[32 * r : 32 * (r + 1), OFF_W3P + q * 32 + 4 * q + r] = W3[:, 0]
    blob[:, OFF_B2P] = np.tile(b2, 4)
    return dict(blob=np.ascontiguousarray(blob))


def _greedy_assign(sim):
    """Sequential greedy matching identical to the reference scan."""
    scores = sim.astype(np.float32).copy()
    assign = np.empty(N_SAT, np.int64)
    for i in range(N_SAT):
        j = int(np.argmax(scores[i]))
        assign[i] = j
        scores[:, j] = -np.inf
    return assign


def kernel(sat_regions, uav_regions, W1, b1, W2, b2, W3, b3):
    global _CACHED_NC
    if _CACHED_NC is None:
        _CACHED_NC = _build_nc()
    nc = _CACHED_NC

    in_maps = []
    for k in range(N_CORES):
        shard = sat_regions[k * RPC : (k + 1) * RPC]
        in_maps.append(_prepack(shard, uav_regions, W1, b1, W2, b2, W3))

    res = run_bass_kernel_spmd(nc, in_maps, core_ids=list(range(N_CORES)))
    global LAST_RESULT
    LAST_RESULT = res
    sim = np.empty((N_SAT, N_UAV), np.float32)
    for k in range(N_CORES):
        la = res.results[k]["lout_all"]  # [32, 9*512]
        for n in range(N_GROUPS * N_CHUNKS):
            g, c = divmod(n, N_CHUNKS)
            sim[
                k * RPC + g * GROUP * 4 : k * RPC + (g + 1) * GROUP * 4,
                c * CHUNK : (c + 1) * CHUNK,
            ] = la[:, n * CHUNK : (n + 1) * CHUNK]

    assign = _greedy_assign(sim)
    out = np.stack([sat_regions, uav_regions[assign]], axis=1)
    return np.ascontiguousarray(out, dtype=np.float32)

